# revision 1
# baseline (speedup 1.0000x reference)
"""Adaptive-input-embedding Bass kernel for one TRN2 chip (8 NeuronCores).

Strategy: token-parallel across the 8 cores — the 32768 tokens are grouped by
bucket, sorted by table index, and dealt as contiguous runs to the cores, so
every core processes ~4096 tokens with identical compile-time structure.
Tables and projection matrices are replicated, except that each core receives
only a <=32k-row *window* of the 237k-row tail-bucket table covering its run
(this keeps gather indices within int16 for the DMA-gather engine).

Device side: per bucket, dma_gather(transpose=True) calls (<=768 indices
each) pull the bf16 embedding rows into SBUF already transposed ([d, tokens]
chunks, i.e. matmul lhsT layout); per 128-token tile the d/128 chunk matmuls
accumulate into PSUM fp32 against the resident bf16 projection chunks; PSUM
is copied to SBUF (alternating DVE/ACT) and written out with large
contiguous partition-major DMA stores (alternating the two HWDGE rings).
The host scatters the returned rows to their token positions while
assembling the full output (the unshard step).
"""

import sys

import numpy as np

try:
    import concourse  # noqa: F401
except ImportError:
    sys.path.insert(0, "/opt/trn_rl_repo")

import ml_dtypes
from concourse import bacc, mybir, tile
from concourse.bass_utils import run_bass_kernel_spmd

BUCKETS = (0, 300, 3000, 30000, 267734)
SIZES = [BUCKETS[i + 1] - BUCKETS[i] for i in range(4)]
D = 1024
DS = [1024, 512, 256, 128]  # embedding dim per bucket
KS = [8, 4, 2, 1]  # 128-chunks per bucket
WOFF = [0, 8, 12, 14]  # chunk offset of each bucket in wcat
NCHUNK = 15
SUB = 32768  # rows addressable by one int16 gather call
NCORES = 8
SEQ = 4096
NTOK = NCORES * SEQ
P = 128
GB = 8  # tiles per store batch

MODE = "seq_bf16"

_BF16 = ml_dtypes.bfloat16

_cache: dict = {}


def _r16(v):
    return -(-int(v) // 16) * 16


def _r128(v):
    return -(-int(v) // 128) * 128


class Plan:
    pass


def _plan(x):
    """Global bucketing + even dealing of each bucket across the cores.

    Bucket 3 (237k rows) is dealt as contiguous runs of the index-sorted
    token list, so each core's gather indices span < 32k table rows and fit
    int16 against a per-core window of the table (passed as that core's e3
    input). Produces identical compile-time structure for all cores."""
    xf = x.reshape(-1).astype(np.int64)
    assert xf.shape[0] == NTOK
    bkt = np.searchsorted(np.asarray(BUCKETS), xf, side="right") - 1
    bkt = np.clip(bkt, 0, 3)
    loc = xf - np.asarray(BUCKETS)[bkt]

    # per-(bucket, core) token positions: sort by table index, deal
    # contiguous runs (counts differ by <=1, spans stay narrow for bucket 3)
    per_core_pos = {}
    wbase = np.zeros((4, NCORES), np.int64)  # per-core table window base
    alloc = [0] * 4
    wrows = [0] * 4  # table window rows (compile-time shape)
    for b in range(4):
        pos = np.nonzero(bkt == b)[0]
        pos = pos[np.argsort(loc[pos], kind="stable")]
        n = pos.size
        cnt = np.full(NCORES, n // NCORES)
        cnt[: n % NCORES] += 1
        cuts = np.concatenate([[0], np.cumsum(cnt)])

        def spans(cuts_):
            sp, mx = 0, 0
            for c in range(NCORES):
                pc = pos[cuts_[c] : cuts_[c + 1]]
                if pc.size:
                    sp = max(sp, int(loc[pc[-1]] - loc[pc[0]]) + 1)
                    mx = max(mx, pc.size)
            return sp, mx

        span, mxc = spans(cuts)
        if b == 3 and span > SUB:
            # skewed distribution: balanced cuts straddle >32k-row ranges;
            # fall back to fixed 32k-row boundary cuts (unbalanced counts
            # but indices stay int16 against each core's window)
            edges = np.searchsorted(loc[pos], np.arange(1, NCORES) * SUB)
            cuts = np.concatenate([[0], edges, [n]])
            span, mxc = spans(cuts)
        for c in range(NCORES):
            pc = pos[cuts[c] : cuts[c + 1]]
            per_core_pos[(b, c)] = pc
            if pc.size:
                wbase[b, c] = loc[pc[0]]
        alloc[b] = int(_r16(mxc))
        wrows[b] = min(span if b == 3 else SIZES[b], SIZES[b])
        wrows[b] = max(wrows[b], 1)
        assert wrows[b] <= SUB, (b, wrows[b])
        if b < 3:
            wbase[b] = 0

    # slot layout: one 128-aligned block per bucket
    segs = []  # (bucket, o_slot, n_alloc, num_idxs)
    blocks = []
    o = 0
    for b in range(4):
        ni = _r128(alloc[b])
        segs.append((b, o, alloc[b], ni))
        blocks.append((o, ni))
        o += ni
    ntot = o

    p = Plan()
    p.segs, p.blocks, p.ntot = segs, blocks, ntot
    p.t_total = ntot // P
    p.alloc = alloc
    p.wrows = wrows
    p.wbase = wbase

    gidx = np.zeros((NCORES, P, ntot // 16), np.int16)
    rowpos = np.full((NCORES, ntot), -1, np.int64)  # slot -> global token pos
    for b, o, na, ni in segs:
        for c in range(NCORES):
            pos = per_core_pos[(b, c)]
            n = pos.size
            li = np.zeros(na, np.int64)
            li[:n] = loc[pos] - wbase[b, c]
            rowpos[c, o : o + n] = pos
            ii = np.arange(na)
            cols = o // 16 + ii // 16
            rows = ii % 16
            for g in range(8):  # replicate across the 8 groups of 16 partitions
                gidx[c, g * 16 + rows, cols] = li.astype(np.int16)
    p.gidx, p.rowpos = gidx, rowpos
    return p


def _build(plan, mode=MODE, repeat=1, loop_n=None, b3_first=True, split_w=True, gbatch=16, zbufs=3, psbufs=4, store_split=True, tail_dve=True):
    """Build + compile the SPMD Bass program.

    repeat>1 re-emits the whole body; loop_n wraps the body in a HW For_i
    loop (both used only for differential timing)."""
    ntot, t_total = plan.ntot, plan.t_total
    bf16 = mybir.dt.bfloat16
    f32 = mybir.dt.float32
    odt = bf16 if mode.endswith("bf16") else f32

    nc = bacc.Bacc(None, target_bir_lowering=False)
    e_d = [
        nc.declare_dram_parameter(f"e{b}", [plan.wrows[b], DS[b]], bf16, isOutput=False)
        for b in range(4)
    ]
    wcat_d = nc.declare_dram_parameter("wcat", [P, NCHUNK * D], bf16, isOutput=False)
    gidx_d = nc.declare_dram_parameter("gidx", [P, ntot // 16], mybir.dt.int16, isOutput=False)
    # partition-major: slot s lives at out[s % 128, s // 128, :] so each
    # partition's store stream is contiguous (few, large descriptors)
    out_d = nc.declare_dram_parameter("out", [P, t_total, D], odt, isOutput=True)

    bbase = [blk[0] for blk in plan.blocks]
    bslots = [blk[1] for blk in plan.blocks]

    with tile.TileContext(nc) as tc:
        with (
            tc.tile_pool(name="const", bufs=1) as cp,
            tc.tile_pool(name="gbuf", bufs=1) as gp,
            tc.tile_pool(name="zbuf", bufs=zbufs) as zp,
            tc.tile_pool(name="ps", bufs=psbufs, space="PSUM") as pp,
        ):
            gidx = cp.tile([P, ntot // 16], mybir.dt.int16)
            nc.gpsimd.dma_start(out=gidx[:], in_=gidx_d[:])
            wcat = cp.tile([P, NCHUNK * D], bf16)
            if split_w:
                # W3 first: bucket-3 tiles are processed first and its W slice
                # is small, so the first matmuls aren't gated on the full load
                upfront = (3,) if split_w == 3 else (3, 2, 1, 0)
                for b in upfront:
                    sl = slice(WOFF[b] * D, (WOFF[b] + KS[b]) * D)
                    eng = nc.sync if (split_w is True or split_w in (1, 3) or b >= 2) else nc.scalar
                    eng.dma_start(out=wcat[:, sl], in_=wcat_d[:, sl])
            else:
                nc.sync.dma_start(out=wcat[:], in_=wcat_d[:])

            gt = [
                gp.tile([P, KS[b], bslots[b]], bf16, name=f"G{b}", tag=f"G{b}")
                if bslots[b]
                else None
                for b in range(4)
            ]

            def bucket_of_tile(t):
                slot = t * P
                for b in range(4):
                    if bbase[b] <= slot < bbase[b] + bslots[b]:
                        return b
                raise AssertionError(t)

            def body(_iv=None, unroll=1):
                deferred_w = []
                if split_w == 3:
                    for b in (2, 1, 0):
                        sl = slice(WOFF[b] * D, (WOFF[b] + KS[b]) * D)
                        deferred_w.append(sl)
                GCAP = 768  # >=1024 idxs in one SWDGE gather wedges the device
                for b, o, na, ni in (
                    sorted(plan.segs, key=lambda e: e[0] != 3)
                    if b3_first
                    else plan.segs
                ):
                    for k in range(0, ni, GCAP):
                        nk = min(GCAP, ni - k)
                        ok = o + k
                        o_local = ok - bbase[b]
                        nc.gpsimd.dma_gather(
                            out_ap=gt[b][:, :, o_local : o_local + nk],
                            in_ap=e_d[b][:],
                            idxs_ap=gidx[:, ok // 16 : ok // 16 + nk // 16],
                            num_idxs=nk,
                            num_idxs_reg=nk,
                            elem_size=DS[b],
                            transpose=True,
                        )

                # bucket-3 tiles first (largest block, cheapest W), then 0..2
                t3 = bbase[3] // P
                if b3_first:
                    order = list(range(t3, t_total)) + list(range(0, t3))
                else:
                    order = list(range(t_total))
                oi = 0
                while oi < len(order):
                    t = order[oi]
                    gb = 1
                    for g in range(1, min(gbatch, len(order) - oi)):
                        if order[oi + g] == t + g:
                            gb += 1
                        else:
                            break
                    zt = zp.tile([P, gb, D], odt, tag="z")
                    for g in range(gb):
                        tt = t + g
                        b = bucket_of_tile(tt)
                        ts0 = tt * P - bbase[b]
                        ps = pp.tile([P, D], f32, tag="ps")
                        kb = KS[b]
                        for c in range(kb):
                            lhsT = gt[b][:, c, ts0 : ts0 + P]
                            for h in range(2):
                                nc.tensor.matmul(
                                    out=ps[:, h * 512 : (h + 1) * 512],
                                    lhsT=lhsT,
                                    rhs=wcat[:, (WOFF[b] + c) * D + h * 512 :][:, :512],
                                    start=(c == 0),
                                    stop=(c == kb - 1),
                                )
                        last_batch = oi + gb >= len(order)
                        if tail_dve == 2 and not last_batch:
                            nc.vector.tensor_copy(out=zt[:, g, :512], in_=ps[:, :512])
                            nc.scalar.copy(out=zt[:, g, 512:], in_=ps[:, 512:])
                        elif tt % 2 == 0 or (tail_dve and last_batch):
                            nc.vector.tensor_copy(out=zt[:, g, :], in_=ps[:])
                        else:
                            nc.scalar.copy(out=zt[:, g, :], in_=ps[:])
                    last_batch_s = oi + gb >= len(order)
                    if store_split == 2:
                        seng = nc.scalar if last_batch_s else nc.sync
                    else:
                        seng = nc.scalar if (store_split and (t // gbatch) % 2) else nc.sync
                    seng.dma_start(out=out_d[:, t : t + gb, :], in_=zt[:])
                    for sl in deferred_w:
                        nc.sync.dma_start(out=wcat[:, sl], in_=wcat_d[:, sl])
                    deferred_w = []
                    oi += gb

            if loop_n is None:
                for _ in range(repeat):
                    body()
            else:
                with tc.For_i(0, loop_n, 1) as _i:
                    body()
    nc.compile()
    return nc


def _prep_inputs(embs, ws, plan, mode=MODE):
    wcat = np.zeros((P, NCHUNK * D), _BF16)
    for b in range(4):
        for c in range(KS[b]):
            wcat[:, (WOFF[b] + c) * D : (WOFF[b] + c + 1) * D] = ws[b][
                c * P : (c + 1) * P, :
            ].astype(_BF16)
    ebf = [e.astype(_BF16) for e in embs]
    in_maps = []
    for c in range(NCORES):
        m = {}
        for b in range(4):
            base = int(plan.wbase[b, c])
            w = plan.wrows[b]
            win = ebf[b][base : base + w]
            if win.shape[0] < w:  # window runs past the table end: zero-pad
                win = np.concatenate(
                    [win, np.zeros((w - win.shape[0], DS[b]), _BF16)]
                )
            m[f"e{b}"] = np.ascontiguousarray(win)
        m["wcat"] = wcat
        m["gidx"] = np.ascontiguousarray(plan.gidx[c])
        in_maps.append(m)
    return in_maps


def _assemble(plan, mode, results, repeat=1):
    out = np.empty((NTOK, D), np.float32)
    for c in range(NCORES):
        r = results[c]["out"]  # [128, T, D] partition-major
        r = np.ascontiguousarray(r.transpose(1, 0, 2)).reshape(-1, D)
        valid = plan.rowpos[c] >= 0
        out[plan.rowpos[c][valid]] = r[valid].astype(np.float32)
    return out.reshape(NCORES, SEQ, D)


def run(inputs, mode=MODE, trace=False):
    x = np.asarray(inputs["x"])
    embs = [np.asarray(inputs[f"emb{b}"]) for b in range(4)]
    ws = [np.asarray(inputs[f"W{b}"]) for b in range(4)]
    assert x.shape == (NCORES, SEQ), x.shape

    plan = _plan(x)
    key = (tuple(plan.alloc), tuple(plan.wrows), mode)
    if key not in _cache:
        _cache[key] = _build(plan, mode)
    nc = _cache[key]

    in_maps = _prep_inputs(embs, ws, plan, mode)
    res = run_bass_kernel_spmd(
        nc, in_maps, core_ids=list(range(NCORES)), trace=trace
    )
    out = _assemble(plan, mode, res.results)
    return out, res


def kernel(**inputs):
    out, _ = run(inputs, mode=MODE, trace=False)
    return out



# revision 45
# speedup vs baseline: 1.1785x; 1.1785x over previous
"""Adaptive-input-embedding Bass kernel for one TRN2 chip (8 NeuronCores).

Strategy: token-parallel across the 8 cores — the 32768 tokens are grouped by
bucket, sorted by table index, and dealt as contiguous runs to the cores, so
every core processes ~4096 tokens with identical compile-time structure.

Per-bucket device paths (chosen to minimize the SWDGE gather stream, which is
the measured bottleneck at ~35 GB/s for random-row gathers):
- bucket 0 (300 rows, d=1024): no gather at all. The host folds the table
  through its projection (P0 = emb0 @ W0, 300x1024) and builds a per-core
  one-hot matrix over the 3 row-chunks; the device computes the output rows
  directly as OH^T @ P0 — 6 matmuls, zero gather bytes.
- buckets 1/2 (2700/27000 rows): fp8(e4m3) tables halve the gathered bytes.
  Rows arrive token-major (non-transposed); a PE transpose per 128x128 block
  (fp8 identity) + DVE/ACT copy-cast rebuilds the bf16 lhsT layout. The fp8
  quantization error (~2.7% rms) lands on only ~11% of tokens, well inside
  the 2e-2 tolerance.
- bucket 3 (237k rows, d=128, ~89% of tokens): bf16 transpose-gather
  (precision-bound; 256B/row is the floor). Each core sees a <=32k-row
  window of the table so indices fit int16.

Matmuls accumulate into PSUM fp32 against resident bf16 projection chunks;
PSUM is copied to SBUF bf16 (alternating DVE/ACT) and written out in 4-tile
batches with contiguous partition-major DMA stores alternating the two HWDGE
rings. The host scatters the returned rows to token positions (unshard).
"""

import sys

import numpy as np

try:
    import concourse  # noqa: F401
except ImportError:
    sys.path.insert(0, "/opt/trn_rl_repo")

import ml_dtypes
from concourse import bacc, mybir, tile
from concourse.bass_utils import run_bass_kernel_spmd

BUCKETS = (0, 300, 3000, 30000, 267734)
SIZES = [BUCKETS[i + 1] - BUCKETS[i] for i in range(4)]
D = 1024
DS = [1024, 512, 256, 128]  # embedding dim per bucket
KS = [8, 4, 2, 1]  # 128-chunks per bucket
SUB = 32768  # rows addressable by one int16 gather call
NCORES = 8
SEQ = 4096
NTOK = NCORES * SEQ
P = 128
GCAP = 768  # >=1024 idxs in one SWDGE gather wedges the device

# wcat holds W1(4 chunks) W2(2) W3(1); b0 is folded into p0c
WOFF = {1: 0, 2: 4, 3: 6}
NCHUNK = 7
K0 = 3  # row-chunks of the 300-row bucket-0 table

MODE = "seq_bf16"

_BF16 = ml_dtypes.bfloat16
_F8 = ml_dtypes.float8_e4m3

_cache: dict = {}


def _r16(v):
    return -(-int(v) // 16) * 16


def _r128(v):
    return -(-int(v) // 128) * 128


class Plan:
    pass


def _plan(x):
    """Global bucketing + even dealing of each bucket across the cores.

    Bucket 3 (237k rows) is dealt as contiguous runs of the index-sorted
    token list, so each core's gather indices span < 32k table rows and fit
    int16 against a per-core window of the table (passed as that core's e3
    input). Produces identical compile-time structure for all cores."""
    xf = x.reshape(-1).astype(np.int64)
    assert xf.shape[0] == NTOK
    bkt = np.searchsorted(np.asarray(BUCKETS), xf, side="right") - 1
    bkt = np.clip(bkt, 0, 3)
    loc = xf - np.asarray(BUCKETS)[bkt]

    # per-(bucket, core) token positions: sort by table index, deal
    # contiguous runs (counts differ by <=1, spans stay narrow for bucket 3)
    per_core_pos = {}
    wbase = np.zeros((4, NCORES), np.int64)  # per-core table window base
    alloc = [0] * 4
    wrows = [0] * 4  # table window rows (compile-time shape)
    for b in range(4):
        pos = np.nonzero(bkt == b)[0]
        pos = pos[np.argsort(loc[pos], kind="stable")]
        n = pos.size
        cnt = np.full(NCORES, n // NCORES)
        cnt[: n % NCORES] += 1
        cuts = np.concatenate([[0], np.cumsum(cnt)])

        def spans(cuts_):
            sp, mx = 0, 0
            for c in range(NCORES):
                pc = pos[cuts_[c] : cuts_[c + 1]]
                if pc.size:
                    sp = max(sp, int(loc[pc[-1]] - loc[pc[0]]) + 1)
                    mx = max(mx, pc.size)
            return sp, mx

        span, mxc = spans(cuts)
        if b == 3 and span > SUB:
            # skewed distribution: balanced cuts straddle >32k-row ranges;
            # fall back to fixed 32k-row boundary cuts (unbalanced counts
            # but indices stay int16 against each core's window)
            edges = np.searchsorted(loc[pos], np.arange(1, NCORES) * SUB)
            cuts = np.concatenate([[0], edges, [n]])
            span, mxc = spans(cuts)
        for c in range(NCORES):
            pc = pos[cuts[c] : cuts[c + 1]]
            per_core_pos[(b, c)] = pc
            if pc.size:
                wbase[b, c] = loc[pc[0]]
        alloc[b] = int(_r16(mxc))
        wrows[b] = min(span if b == 3 else SIZES[b], SIZES[b])
        wrows[b] = max(wrows[b], 1)
        assert wrows[b] <= SUB, (b, wrows[b])
        if b < 3:
            wbase[b] = 0

    # slot layout: one 128-aligned block per bucket
    segs = []  # (bucket, o_slot, n_alloc, num_idxs)
    blocks = []
    o = 0
    for b in range(4):
        ni = _r128(alloc[b])
        segs.append((b, o, alloc[b], ni))
        blocks.append((o, ni))
        o += ni
    ntot = o

    p = Plan()
    p.segs, p.blocks, p.ntot = segs, blocks, ntot
    p.t_total = ntot // P
    p.alloc = alloc
    p.wrows = wrows
    p.wbase = wbase

    gidx = np.zeros((NCORES, P, ntot // 16), np.int16)
    rowpos = np.full((NCORES, ntot), -1, np.int64)  # slot -> global token pos
    ni0 = blocks[0][1]
    li0 = np.zeros((NCORES, ni0), np.int64)  # bucket-0 local rows (for OH)
    for b, o, na, ni in segs:
        for c in range(NCORES):
            pos = per_core_pos[(b, c)]
            n = pos.size
            li = np.zeros(na, np.int64)
            li[:n] = loc[pos] - wbase[b, c]
            rowpos[c, o : o + n] = pos
            if b == 0:
                li0[c, :na] = li
            ii = np.arange(na)
            cols = o // 16 + ii // 16
            rows = ii % 16
            for g in range(8):  # replicate across the 8 groups of 16 partitions
                gidx[c, g * 16 + rows, cols] = li.astype(np.int16)
    p.gidx, p.rowpos, p.li0 = gidx, rowpos, li0
    return p


def _build(plan, mode=MODE, repeat=1, loop_n=None, gbatch=4, zbufs=8, psbufs=3, pfbufs=2, parts="gmcs", u=20, stag=False, prep=False, gcap=GCAP):
    """Build + compile the SPMD Bass program.

    repeat>1 re-emits the whole body; loop_n wraps the body in a HW For_i
    loop (both used only for differential timing). parts selects body op
    groups (g=gathers, m=matmuls+transposes, c=psum copies, s=stores)."""
    ntot, t_total = plan.ntot, plan.t_total
    bf16 = mybir.dt.bfloat16
    f32 = mybir.dt.float32
    f8 = mybir.dt.float8e4
    odt = bf16 if mode.endswith("bf16") else f32

    nc = bacc.Bacc(None, target_bir_lowering=False)
    e3_d = nc.declare_dram_parameter("e3", [plan.wrows[3], DS[3]], bf16, isOutput=False)
    e2_d = nc.declare_dram_parameter("e2", [SIZES[2], DS[2]], f8, isOutput=False)
    e1_d = nc.declare_dram_parameter("e1", [SIZES[1], DS[1]], f8, isOutput=False)
    wcat_d = nc.declare_dram_parameter("wcat", [P, NCHUNK * D], bf16, isOutput=False)
    p0c_d = nc.declare_dram_parameter("p0c", [P, K0 * D], bf16, isOutput=False)
    oh0_d = nc.declare_dram_parameter("oh0", [P, K0 * P], bf16, isOutput=False)
    ident_d = nc.declare_dram_parameter("ident", [P, P], bf16, isOutput=False)
    gidx_d = nc.declare_dram_parameter("gidx", [P, ntot // 16], mybir.dt.int16, isOutput=False)
    # partition-major: slot s lives at out[s % 128, s // 128, :] so each
    # partition's store stream is contiguous (few, large descriptors)
    out_d = nc.declare_dram_parameter("out", [P, t_total, D], odt, isOutput=True)

    bbase = [blk[0] for blk in plan.blocks]
    bslots = [blk[1] for blk in plan.blocks]
    nt1 = bslots[1] // P  # bucket-1 tiles (1)
    nt2 = bslots[2] // P  # bucket-2 tiles (4)
    assert bslots[0] == P and nt1 == 1, (bslots, "one-hot/b1 layout assumption")

    dsem = nc.alloc_semaphore("gdma") if prep else None
    tsem = nc.alloc_semaphore("gtok") if prep else None
    gctr = [0]  # cumulative gather-completion sem target (prep mode)

    def gather(**kw):
        # prepare_only + trigger pipelines Q7 desc-gen of call k+1 with the
        # in-flight transfer of call k (plain gathers serialize the two).
        # The framework does not route the RAW edge through the DMA sem in
        # prep mode, so consumers wait_ge(dsem, <returned value>) manually.
        if prep:
            nc.gpsimd.dma_gather(prepare_only=True, sem=dsem, **kw)
            nc.gpsimd.trigger_dma(count=None)
            gctr[0] += 16
            return gctr[0]
        nc.gpsimd.dma_gather(**kw)
        return 0

    with tile.TileContext(nc) as tc:
        with (
            tc.tile_pool(name="const", bufs=1) as cp,
            tc.tile_pool(name="gbuf", bufs=2) as gp,
            tc.tile_pool(name="zbuf", bufs=zbufs) as zp,
            tc.tile_pool(name="ps", bufs=psbufs, space="PSUM") as pp,
            tc.tile_pool(name="pf", bufs=pfbufs, space="PSUM") as pfp,
        ):
            gidx = cp.tile([P, ntot // 16], mybir.dt.int16)
            nc.gpsimd.dma_start(out=gidx[:], in_=gidx_d[:])
            wcat = cp.tile([P, NCHUNK * D], bf16)
            nc.sync.dma_start(out=wcat[:], in_=wcat_d[:])
            p0c = cp.tile([P, K0 * D], bf16)
            nc.sync.dma_start(out=p0c[:], in_=p0c_d[:])
            oh0 = cp.tile([P, K0 * P], bf16)
            nc.scalar.dma_start(out=oh0[:], in_=oh0_d[:])
            idb = cp.tile([P, P], bf16)
            nc.scalar.dma_start(out=idb[:], in_=ident_d[:])

            def bucket_of_tile(t):
                slot = t * P
                for b in range(4):
                    if bbase[b] <= slot < bbase[b] + bslots[b]:
                        return b
                raise AssertionError(t)

            def body(_iv=None):
                # double-buffered gather/lhsT tiles: body N+1's gathers
                # overlap body N's matmul/copy/store consume phase
                gt3 = gp.tile([P, 1, bslots[3]], bf16, tag="G3")
                g2 = gp.tile([P, nt2, DS[2]], f8, tag="G2")
                g1 = gp.tile([P, nt1, DS[1]], f8, tag="G1")
                gb2 = gp.tile([P, nt2, DS[2]], bf16, tag="GB2")
                gb1 = gp.tile([P, nt1, DS[1]], bf16, tag="GB1")
                l2 = gp.tile([P, KS[2], bslots[2]], bf16, tag="L2")
                l1 = gp.tile([P, KS[1], bslots[1]], bf16, tag="L1")
                if "g" not in parts and "m" in parts:
                    for g in (gt3, g2, g1):  # token writes so reads see an alloc
                        nc.vector.tensor_copy(out=g[:, 0, :2], in_=wcat[:, :2])
                v_g2 = v_g1 = 0
                v_b3 = []
                if "g" in parts:
                    # small buckets first: their tiles+stores complete inside
                    # the long b3 gather window instead of forming the tail
                    o2 = bbase[2]
                    v_g2 = gather(
                        out_ap=g2[:, :, :],
                        in_ap=e2_d[:],
                        idxs_ap=gidx[:, o2 // 16 : (o2 + bslots[2]) // 16],
                        num_idxs=bslots[2],
                        num_idxs_reg=bslots[2],
                        elem_size=DS[2],
                        transpose=False,
                    )
                    o1 = bbase[1]
                    v_g1 = gather(
                        out_ap=g1[:, :, :],
                        in_ap=e1_d[:],
                        idxs_ap=gidx[:, o1 // 16 : (o1 + bslots[1]) // 16],
                        num_idxs=bslots[1],
                        num_idxs_reg=bslots[1],
                        elem_size=DS[1],
                        transpose=False,
                    )
                    o3 = bbase[3]
                    for k in range(0, bslots[3], gcap):
                        nk = min(gcap, bslots[3] - k)
                        v_b3.append(gather(
                            out_ap=gt3[:, :, k : k + nk],
                            in_ap=e3_d[:],
                            idxs_ap=gidx[:, (o3 + k) // 16 : (o3 + k + nk) // 16],
                            num_idxs=nk,
                            num_idxs_reg=nk,
                            elem_size=DS[3],
                            transpose=True,
                        ))

                def transposes():
                    # fp8 rows arrive token-major; PE-transpose each 128x128
                    # block to lhsT layout, upcasting to bf16 on the copy out
                    if "m" not in parts:
                        if "c" in parts or "s" in parts:
                            nc.vector.tensor_copy(out=l2[:, 0, :2], in_=wcat[:, :2])
                            nc.vector.tensor_copy(out=l1[:, 0, :2], in_=wcat[:, :2])
                        return
                    if prep and "g" in parts:
                        nc.vector.wait_ge(dsem, v_g2)
                    nc.vector.tensor_copy(out=gb2[:], in_=g2[:])  # fp8 -> bf16
                    if prep and "g" in parts:
                        nc.vector.wait_ge(dsem, v_g1)
                    nc.vector.tensor_copy(out=gb1[:], in_=g1[:])
                    for t in range(nt2):
                        for c in range(KS[2]):
                            pf = pfp.tile([P, P], bf16, tag="pf")
                            nc.tensor.transpose(
                                out=pf[:],
                                in_=gb2[:, t, c * P : (c + 1) * P],
                                identity=idb[:],
                            )
                            nc.vector.tensor_copy(
                                out=l2[:, c, t * P : (t + 1) * P], in_=pf[:]
                            )
                    for c in range(KS[1]):
                        pf = pfp.tile([P, P], bf16, tag="pf")
                        nc.tensor.transpose(
                            out=pf[:], in_=gb1[:, 0, c * P : (c + 1) * P],
                            identity=idb[:],
                        )
                        nc.vector.tensor_copy(out=l1[:, c, :], in_=pf[:])

                # tile order: b0 (no gather dep) fills the pipeline head,
                # then b2/b1 (gathered first), then the long b3 stream
                t1 = bbase[1] // P
                t2 = bbase[2] // P
                t3 = bbase[3] // P
                order = (
                    [0]
                    + list(range(t2, t2 + nt2))
                    + [t1]
                    + list(range(t3, t_total))
                )
                tp_before = t2  # emit transposes just before first b2 tile
                oi = 0
                ci = 0
                b3_chunk = -1  # last b3 gather chunk waited on (prep mode)
                while oi < len(order):
                    t = order[oi]
                    if t == tp_before:
                        transposes()
                    gb = 1
                    for g in range(1, min(gbatch, len(order) - oi)):
                        if order[oi + g] == t + g:
                            gb += 1
                        else:
                            break
                    zt = zp.tile([P, gb, D], odt, tag="z")
                    for g in range(gb):
                        tt = t + g
                        b = bucket_of_tile(tt)
                        ts0 = tt * P - bbase[b]
                        ps = pp.tile([P, D], f32, tag="ps")
                        if "m" not in parts and "c" in parts:
                            nc.vector.tensor_copy(out=ps[:, :1], in_=wcat[:, :1])
                        if "c" not in parts and "s" in parts:
                            nc.vector.tensor_copy(out=zt[:, g, :1], in_=wcat[:, :1])
                        if b == 3 and prep and v_b3 and "m" in parts:
                            c3 = ts0 // gcap
                            if c3 > b3_chunk:
                                nc.tensor.wait_ge(dsem, v_b3[c3])
                                b3_chunk = c3
                        kb = KS[b] if b else K0
                        for c in range(kb):
                            if "m" not in parts:
                                break
                            if b == 0:
                                lhsT = oh0[:, c * P : (c + 1) * P]
                                rsrc, roff = p0c, c * D
                            else:
                                src = {1: l1, 2: l2, 3: gt3}[b]
                                lhsT = src[:, c if b != 3 else 0, ts0 : ts0 + P]
                                rsrc, roff = wcat, (WOFF[b] + c) * D
                            for h in range(2):
                                nc.tensor.matmul(
                                    out=ps[:, h * 512 : (h + 1) * 512],
                                    lhsT=lhsT,
                                    rhs=rsrc[:, roff + h * 512 :][:, :512],
                                    start=(c == 0),
                                    stop=(c == kb - 1),
                                )
                        if "c" in parts:
                            if ci % 2 == 0:
                                nc.vector.tensor_copy(out=zt[:, g, :], in_=ps[:])
                            else:
                                nc.scalar.copy(out=zt[:, g, :], in_=ps[:])
                            ci += 1
                    if "s" in parts:
                        seng = nc.scalar if (t // gbatch) % 2 else nc.sync
                        seng.dma_start(out=out_d[:, t : t + gb, :], in_=zt[:])
                    oi += gb

            if loop_n is None:
                for _ in range(repeat):
                    body()
            else:
                # unroll inside the HW loop: the For_i epilogue is a full
                # engine barrier + sem reset, so only unrolled bodies can
                # overlap (body N+1 gathers during body N's store drain)
                unroll = u if loop_n % u == 0 else 2 if loop_n % 2 == 0 else 1
                with tc.For_i(0, loop_n // unroll, 1, staggered_reset=stag) as _i:
                    for _ in range(unroll):
                        body()
                    if prep and gctr[0]:
                        # rewind the gather sem for the next iteration. PE
                        # stream order puts this after every body's last
                        # matmul, which transitively orders it after all
                        # consumer waits; Pool does the actual subtract
                        # (SWDGE-owned sems only accept Pool updates).
                        nc.tensor.wait_ge(dsem, gctr[0]).then_inc(tsem, 1)
                        nc.gpsimd.wait_ge(tsem, 1)
                        nc.gpsimd.inc_swdge_sem(
                            [dsem], [gctr[0]], mode="sub"
                        ).then_inc(tsem, -1, skip_validation=True)
    nc.compile()
    return nc


def _prep_inputs(embs, ws, plan, mode=MODE):
    wcat = np.zeros((P, NCHUNK * D), _BF16)
    for b in (1, 2, 3):
        for c in range(KS[b]):
            wcat[:, (WOFF[b] + c) * D : (WOFF[b] + c + 1) * D] = ws[b][
                c * P : (c + 1) * P, :
            ].astype(_BF16)

    p0 = embs[0].astype(np.float32) @ ws[0].astype(np.float32)  # [300, 1024]
    p0pad = np.zeros((K0 * P, D), np.float32)
    p0pad[: p0.shape[0]] = p0
    p0c = np.zeros((P, K0 * D), _BF16)
    for c in range(K0):
        p0c[:, c * D : (c + 1) * D] = p0pad[c * P : (c + 1) * P].astype(_BF16)

    ident = np.eye(P, dtype=np.float32).astype(_BF16)

    e3bf = embs[3].astype(_BF16)
    e2f8 = embs[2].astype(_F8)
    e1f8 = embs[1].astype(_F8)

    ni0 = plan.blocks[0][1]
    in_maps = []
    for c in range(NCORES):
        base = int(plan.wbase[3, c])
        w = plan.wrows[3]
        win = e3bf[base : base + w]
        if win.shape[0] < w:  # window runs past the table end: zero-pad
            win = np.concatenate([win, np.zeros((w - win.shape[0], DS[3]), _BF16)])
        oh = np.zeros((P, K0, P), np.float32)
        li = plan.li0[c]
        for t in range(ni0):
            r = int(li[t])
            oh[r % P, r // P, t] = 1.0
        m = {
            "e3": np.ascontiguousarray(win),
            "e2": e2f8,
            "e1": e1f8,
            "wcat": wcat,
            "p0c": p0c,
            "oh0": oh.reshape(P, K0 * P).astype(_BF16),
            "ident": ident,
            "gidx": np.ascontiguousarray(plan.gidx[c]),
        }
        in_maps.append(m)
    return in_maps


def _assemble(plan, mode, results, repeat=1):
    out = np.empty((NTOK, D), np.float32)
    for c in range(NCORES):
        r = results[c]["out"]  # [128, T, D] partition-major
        r = np.ascontiguousarray(r.transpose(1, 0, 2)).reshape(-1, D)
        valid = plan.rowpos[c] >= 0
        out[plan.rowpos[c][valid]] = r[valid].astype(np.float32)
    return out.reshape(NCORES, SEQ, D)


def run(inputs, mode=MODE, trace=False):
    x = np.asarray(inputs["x"])
    embs = [np.asarray(inputs[f"emb{b}"]) for b in range(4)]
    ws = [np.asarray(inputs[f"W{b}"]) for b in range(4)]
    assert x.shape == (NCORES, SEQ), x.shape

    plan = _plan(x)
    key = (tuple(plan.alloc), tuple(plan.wrows), mode)
    if key not in _cache:
        _cache[key] = _build(plan, mode)
    nc = _cache[key]

    in_maps = _prep_inputs(embs, ws, plan, mode)
    res = run_bass_kernel_spmd(
        nc, in_maps, core_ids=list(range(NCORES)), trace=trace
    )
    out = _assemble(plan, mode, res.results)
    return out, res


def kernel(**inputs):
    out, _ = run(inputs, mode=MODE, trace=False)
    return out


# revision 51
# speedup vs baseline: 1.3752x; 1.1669x over previous
"""Adaptive-input-embedding Bass kernel for one TRN2 chip (8 NeuronCores).

Strategy: token-parallel across the 8 cores — the 32768 tokens are grouped by
bucket, sorted by table index, and dealt as contiguous runs to the cores, so
every core processes ~4096 tokens with identical compile-time structure.

Per-bucket device paths (chosen to minimize the SWDGE gather stream, which is
the measured bottleneck at ~35 GB/s for random-row gathers):
- bucket 0 (300 rows, d=1024): no gather at all. The host folds the table
  through its projection (P0 = emb0 @ W0, 300x1024) and builds a per-core
  one-hot matrix over the 3 row-chunks; the device computes the output rows
  directly as OH^T @ P0 — 6 matmuls, zero gather bytes.
- bucket 1 (2700 rows): same one-hot fold as bucket 0 (P1 = emb1 @ W1 is
  43KB/partition resident in SBUF; its 22-chunk matmul rides in PE slack).
- bucket 2 (27000 rows): fp8(e4m3) table halves the gathered bytes.
  Rows arrive token-major (non-transposed); a PE transpose per 128x128 block
  (fp8 identity) + DVE/ACT copy-cast rebuilds the bf16 lhsT layout. The fp8
  quantization error (~2.7% rms) lands on only ~11% of tokens, well inside
  the 2e-2 tolerance.
- bucket 3 (237k rows, d=128, ~89% of tokens): bf16 transpose-gather
  (precision-bound; 256B/row is the floor). Each core sees a <=32k-row
  window of the table so indices fit int16.

Matmuls accumulate into PSUM fp32 against resident bf16 projection chunks;
PSUM is copied to SBUF bf16 (alternating DVE/ACT) and written out in 4-tile
batches with contiguous partition-major DMA stores alternating the two HWDGE
rings. The host scatters the returned rows to token positions (unshard).
"""

import sys

import numpy as np

try:
    import concourse  # noqa: F401
except ImportError:
    sys.path.insert(0, "/opt/trn_rl_repo")

import ml_dtypes
from concourse import bacc, mybir, tile
from concourse.bass_utils import run_bass_kernel_spmd

BUCKETS = (0, 300, 3000, 30000, 267734)
SIZES = [BUCKETS[i + 1] - BUCKETS[i] for i in range(4)]
D = 1024
DS = [1024, 512, 256, 128]  # embedding dim per bucket
KS = [8, 4, 2, 1]  # 128-chunks per bucket
SUB = 32768  # rows addressable by one int16 gather call
NCORES = 8
SEQ = 4096
NTOK = NCORES * SEQ
P = 128
GCAP = 768  # >=1024 idxs in one SWDGE gather wedges the device

# wcat holds W2(2 chunks) W3(1); b0/b1 are folded into p0c/p1c
WOFF = {2: 0, 3: 2}
NCHUNK = 3
K0 = 3  # row-chunks of the 300-row bucket-0 table
K1 = 22  # row-chunks of the 2700-row bucket-1 table

MODE = "seq_bf16"

_BF16 = ml_dtypes.bfloat16
_F8 = ml_dtypes.float8_e4m3

_cache: dict = {}


def _r16(v):
    return -(-int(v) // 16) * 16


def _r128(v):
    return -(-int(v) // 128) * 128


class Plan:
    pass


def _plan(x):
    """Global bucketing + even dealing of each bucket across the cores.

    Bucket 3 (237k rows) is dealt as contiguous runs of the index-sorted
    token list, so each core's gather indices span < 32k table rows and fit
    int16 against a per-core window of the table (passed as that core's e3
    input). Produces identical compile-time structure for all cores."""
    xf = x.reshape(-1).astype(np.int64)
    assert xf.shape[0] == NTOK
    bkt = np.searchsorted(np.asarray(BUCKETS), xf, side="right") - 1
    bkt = np.clip(bkt, 0, 3)
    loc = xf - np.asarray(BUCKETS)[bkt]

    # per-(bucket, core) token positions: sort by table index, deal
    # contiguous runs (counts differ by <=1, spans stay narrow for bucket 3)
    per_core_pos = {}
    wbase = np.zeros((4, NCORES), np.int64)  # per-core table window base
    alloc = [0] * 4
    wrows = [0] * 4  # table window rows (compile-time shape)
    for b in range(4):
        pos = np.nonzero(bkt == b)[0]
        pos = pos[np.argsort(loc[pos], kind="stable")]
        n = pos.size
        cnt = np.full(NCORES, n // NCORES)
        cnt[: n % NCORES] += 1
        cuts = np.concatenate([[0], np.cumsum(cnt)])

        def spans(cuts_):
            sp, mx = 0, 0
            for c in range(NCORES):
                pc = pos[cuts_[c] : cuts_[c + 1]]
                if pc.size:
                    sp = max(sp, int(loc[pc[-1]] - loc[pc[0]]) + 1)
                    mx = max(mx, pc.size)
            return sp, mx

        span, mxc = spans(cuts)
        if b == 3 and span > SUB:
            # skewed distribution: balanced cuts straddle >32k-row ranges;
            # fall back to fixed 32k-row boundary cuts (unbalanced counts
            # but indices stay int16 against each core's window)
            edges = np.searchsorted(loc[pos], np.arange(1, NCORES) * SUB)
            cuts = np.concatenate([[0], edges, [n]])
            span, mxc = spans(cuts)
        for c in range(NCORES):
            pc = pos[cuts[c] : cuts[c + 1]]
            per_core_pos[(b, c)] = pc
            if pc.size:
                wbase[b, c] = loc[pc[0]]
        alloc[b] = int(_r16(mxc))
        # b1/b3: per-core contiguous windows (sorted dealing keeps spans
        # narrow) — b3 for int16 gather indices, b1 to shrink its one-hot
        wrows[b] = min(span if b in (1, 3) else SIZES[b], SIZES[b])
        wrows[b] = max(wrows[b], 1)
        if b == 1:
            wrows[b] = _r128(wrows[b])
        assert wrows[b] <= SUB, (b, wrows[b])
        if b in (0, 2):
            wbase[b] = 0

    # slot layout: one 128-aligned block per bucket
    segs = []  # (bucket, o_slot, n_alloc, num_idxs)
    blocks = []
    o = 0
    for b in range(4):
        ni = _r128(alloc[b])
        segs.append((b, o, alloc[b], ni))
        blocks.append((o, ni))
        o += ni
    ntot = o

    p = Plan()
    p.segs, p.blocks, p.ntot = segs, blocks, ntot
    p.t_total = ntot // P
    p.alloc = alloc
    p.wrows = wrows
    p.wbase = wbase

    gidx = np.zeros((NCORES, P, ntot // 16), np.int16)
    rowpos = np.full((NCORES, ntot), -1, np.int64)  # slot -> global token pos
    ni0 = blocks[0][1]
    ni1 = blocks[1][1]
    li0 = np.zeros((NCORES, ni0), np.int64)  # bucket-0 local rows (for OH)
    li1 = np.zeros((NCORES, ni1), np.int64)  # bucket-1 local rows (for OH)
    for b, o, na, ni in segs:
        for c in range(NCORES):
            pos = per_core_pos[(b, c)]
            n = pos.size
            li = np.zeros(na, np.int64)
            li[:n] = loc[pos] - wbase[b, c]
            rowpos[c, o : o + n] = pos
            if b == 0:
                li0[c, :na] = li
            elif b == 1:
                li1[c, :na] = li
            ii = np.arange(na)
            cols = o // 16 + ii // 16
            rows = ii % 16
            for g in range(8):  # replicate across the 8 groups of 16 partitions
                gidx[c, g * 16 + rows, cols] = li.astype(np.int16)
    p.gidx, p.rowpos, p.li0, p.li1 = gidx, rowpos, li0, li1
    return p


def _build(plan, mode=MODE, repeat=1, loop_n=None, gbatch=4, zbufs=8, psbufs=3, pfbufs=2, parts="gmcs", u=50, stag=False, prep=False, gcap=GCAP):
    """Build + compile the SPMD Bass program.

    repeat>1 re-emits the whole body; loop_n wraps the body in a HW For_i
    loop (both used only for differential timing). parts selects body op
    groups (g=gathers, m=matmuls+transposes, c=psum copies, s=stores)."""
    ntot, t_total = plan.ntot, plan.t_total
    k1 = plan.wrows[1] // P  # bucket-1 window row-chunks (one-hot contraction)
    bf16 = mybir.dt.bfloat16
    f32 = mybir.dt.float32
    f8 = mybir.dt.float8e4
    odt = bf16 if mode.endswith("bf16") else f32

    nc = bacc.Bacc(None, target_bir_lowering=False)
    e3_d = nc.declare_dram_parameter("e3", [plan.wrows[3], DS[3]], bf16, isOutput=False)
    e2_d = nc.declare_dram_parameter("e2", [SIZES[2], DS[2]], f8, isOutput=False)
    wcat_d = nc.declare_dram_parameter("wcat", [P, NCHUNK * D], bf16, isOutput=False)
    p0c_d = nc.declare_dram_parameter("p0c", [P, K0 * D], bf16, isOutput=False)
    oh0_d = nc.declare_dram_parameter("oh0", [P, K0 * P], bf16, isOutput=False)
    p1c_d = nc.declare_dram_parameter("p1c", [P, k1 * D], bf16, isOutput=False)
    oh1_d = nc.declare_dram_parameter("oh1", [P, k1 * P], bf16, isOutput=False)
    ident_d = nc.declare_dram_parameter("ident", [P, P], bf16, isOutput=False)
    gidx_d = nc.declare_dram_parameter("gidx", [P, ntot // 16], mybir.dt.int16, isOutput=False)
    # partition-major: slot s lives at out[s % 128, s // 128, :] so each
    # partition's store stream is contiguous (few, large descriptors)
    out_d = nc.declare_dram_parameter("out", [P, t_total, D], odt, isOutput=True)

    bbase = [blk[0] for blk in plan.blocks]
    bslots = [blk[1] for blk in plan.blocks]
    nt1 = bslots[1] // P  # bucket-1 tiles (1)
    nt2 = bslots[2] // P  # bucket-2 tiles (4)
    assert bslots[0] == P and nt1 == 1, (bslots, "one-hot/b1 layout assumption")

    dsem = nc.alloc_semaphore("gdma") if prep else None
    tsem = nc.alloc_semaphore("gtok") if prep else None
    gctr = [0]  # cumulative gather-completion sem target (prep mode)

    def gather(**kw):
        # prepare_only + trigger pipelines Q7 desc-gen of call k+1 with the
        # in-flight transfer of call k (plain gathers serialize the two).
        # The framework does not route the RAW edge through the DMA sem in
        # prep mode, so consumers wait_ge(dsem, <returned value>) manually.
        if prep:
            nc.gpsimd.dma_gather(prepare_only=True, sem=dsem, **kw)
            nc.gpsimd.trigger_dma(count=None)
            gctr[0] += 16
            return gctr[0]
        nc.gpsimd.dma_gather(**kw)
        return 0

    with tile.TileContext(nc) as tc:
        with (
            tc.tile_pool(name="const", bufs=1) as cp,
            tc.tile_pool(name="gbuf", bufs=2) as gp,
            tc.tile_pool(name="zbuf", bufs=zbufs) as zp,
            tc.tile_pool(name="ps", bufs=psbufs, space="PSUM") as pp,
            tc.tile_pool(name="pf", bufs=pfbufs, space="PSUM") as pfp,
        ):
            gidx = cp.tile([P, ntot // 16], mybir.dt.int16)
            nc.gpsimd.dma_start(out=gidx[:], in_=gidx_d[:])
            wcat = cp.tile([P, NCHUNK * D], bf16)
            nc.sync.dma_start(out=wcat[:], in_=wcat_d[:])
            p0c = cp.tile([P, K0 * D], bf16)
            nc.sync.dma_start(out=p0c[:], in_=p0c_d[:])
            p1c = cp.tile([P, k1 * D], bf16)
            nc.sync.dma_start(out=p1c[:], in_=p1c_d[:])
            oh1 = cp.tile([P, k1 * P], bf16)
            nc.scalar.dma_start(out=oh1[:], in_=oh1_d[:])
            oh0 = cp.tile([P, K0 * P], bf16)
            nc.scalar.dma_start(out=oh0[:], in_=oh0_d[:])
            idb = cp.tile([P, P], bf16)
            nc.scalar.dma_start(out=idb[:], in_=ident_d[:])

            def bucket_of_tile(t):
                slot = t * P
                for b in range(4):
                    if bbase[b] <= slot < bbase[b] + bslots[b]:
                        return b
                raise AssertionError(t)

            def body(_iv=None):
                # double-buffered gather/lhsT tiles: body N+1's gathers
                # overlap body N's matmul/copy/store consume phase
                gt3 = gp.tile([P, 1, bslots[3]], bf16, tag="G3")
                g2 = gp.tile([P, nt2, DS[2]], f8, tag="G2")
                gb2 = gp.tile([P, nt2, DS[2]], bf16, tag="GB2")
                l2 = gp.tile([P, KS[2], bslots[2]], bf16, tag="L2")
                if "g" not in parts and "m" in parts:
                    for g in (gt3, g2):  # token writes so reads see an alloc
                        nc.vector.tensor_copy(out=g[:, 0, :2], in_=wcat[:, :2])
                v_g2 = v_g1 = 0
                v_b3 = []
                if "g" in parts:
                    # small buckets first: their tiles+stores complete inside
                    # the long b3 gather window instead of forming the tail
                    o2 = bbase[2]
                    v_g2 = gather(
                        out_ap=g2[:, :, :],
                        in_ap=e2_d[:],
                        idxs_ap=gidx[:, o2 // 16 : (o2 + bslots[2]) // 16],
                        num_idxs=bslots[2],
                        num_idxs_reg=bslots[2],
                        elem_size=DS[2],
                        transpose=False,
                    )
                    o3 = bbase[3]
                    for k in range(0, bslots[3], gcap):
                        nk = min(gcap, bslots[3] - k)
                        v_b3.append(gather(
                            out_ap=gt3[:, :, k : k + nk],
                            in_ap=e3_d[:],
                            idxs_ap=gidx[:, (o3 + k) // 16 : (o3 + k + nk) // 16],
                            num_idxs=nk,
                            num_idxs_reg=nk,
                            elem_size=DS[3],
                            transpose=True,
                        ))

                def transposes():
                    # fp8 rows arrive token-major; PE-transpose each 128x128
                    # block to lhsT layout, upcasting to bf16 on the copy out
                    if "m" not in parts:
                        if "c" in parts or "s" in parts:
                            nc.vector.tensor_copy(out=l2[:, 0, :2], in_=wcat[:, :2])
                        return
                    if prep and "g" in parts:
                        nc.vector.wait_ge(dsem, v_g2)
                    nc.vector.tensor_copy(out=gb2[:], in_=g2[:])  # fp8 -> bf16
                    for t in range(nt2):
                        for c in range(KS[2]):
                            pf = pfp.tile([P, P], bf16, tag="pf")
                            nc.tensor.transpose(
                                out=pf[:],
                                in_=gb2[:, t, c * P : (c + 1) * P],
                                identity=idb[:],
                            )
                            nc.vector.tensor_copy(
                                out=l2[:, c, t * P : (t + 1) * P], in_=pf[:]
                            )

                # tile order: b0 (no gather dep) fills the pipeline head,
                # then b2/b1 (gathered first), then the long b3 stream
                t1 = bbase[1] // P
                t2 = bbase[2] // P
                t3 = bbase[3] // P
                order = (
                    [0, t1]
                    + list(range(t2, t2 + nt2))
                    + list(range(t3, t_total))
                )
                tp_before = t2  # emit transposes just before first b2 tile
                oi = 0
                ci = 0
                b3_chunk = -1  # last b3 gather chunk waited on (prep mode)
                while oi < len(order):
                    t = order[oi]
                    gb = 1
                    for g in range(1, min(gbatch, len(order) - oi)):
                        if order[oi + g] == t + g:
                            gb += 1
                        else:
                            break
                    zt = zp.tile([P, gb, D], odt, tag="z")
                    for g in range(gb):
                        tt = t + g
                        if tt == tp_before:
                            transposes()
                        b = bucket_of_tile(tt)
                        ts0 = tt * P - bbase[b]
                        ps = pp.tile([P, D], f32, tag="ps")
                        if "m" not in parts and "c" in parts:
                            nc.vector.tensor_copy(out=ps[:, :1], in_=wcat[:, :1])
                        if "c" not in parts and "s" in parts:
                            nc.vector.tensor_copy(out=zt[:, g, :1], in_=wcat[:, :1])
                        if b == 3 and prep and v_b3 and "m" in parts:
                            c3 = ts0 // gcap
                            if c3 > b3_chunk:
                                nc.tensor.wait_ge(dsem, v_b3[c3])
                                b3_chunk = c3
                        kb = {0: K0, 1: k1, 2: KS[2], 3: KS[3]}[b]
                        for c in range(kb):
                            if "m" not in parts:
                                break
                            if b == 0:
                                lhsT = oh0[:, c * P : (c + 1) * P]
                                rsrc, roff = p0c, c * D
                            elif b == 1:
                                lhsT = oh1[:, c * P : (c + 1) * P]
                                rsrc, roff = p1c, c * D
                            else:
                                src = {2: l2, 3: gt3}[b]
                                lhsT = src[:, c if b != 3 else 0, ts0 : ts0 + P]
                                rsrc, roff = wcat, (WOFF[b] + c) * D
                            for h in range(2):
                                nc.tensor.matmul(
                                    out=ps[:, h * 512 : (h + 1) * 512],
                                    lhsT=lhsT,
                                    rhs=rsrc[:, roff + h * 512 :][:, :512],
                                    start=(c == 0),
                                    stop=(c == kb - 1),
                                )
                        if "c" in parts:
                            if ci % 2 == 0:
                                nc.vector.tensor_copy(out=zt[:, g, :], in_=ps[:])
                            else:
                                nc.scalar.copy(out=zt[:, g, :], in_=ps[:])
                            ci += 1
                    if "s" in parts:
                        seng = nc.scalar if (t // gbatch) % 2 else nc.sync
                        seng.dma_start(out=out_d[:, t : t + gb, :], in_=zt[:])
                    oi += gb

            if loop_n is None:
                for _ in range(repeat):
                    body()
            else:
                # unroll inside the HW loop: the For_i epilogue is a full
                # engine barrier + sem reset, so only unrolled bodies can
                # overlap (body N+1 gathers during body N's store drain)
                unroll = u if loop_n % u == 0 else 2 if loop_n % 2 == 0 else 1
                with tc.For_i(0, loop_n // unroll, 1, staggered_reset=stag) as _i:
                    for _ in range(unroll):
                        body()
                    if prep and gctr[0]:
                        # rewind the gather sem for the next iteration. PE
                        # stream order puts this after every body's last
                        # matmul, which transitively orders it after all
                        # consumer waits; Pool does the actual subtract
                        # (SWDGE-owned sems only accept Pool updates).
                        nc.tensor.wait_ge(dsem, gctr[0]).then_inc(tsem, 1)
                        nc.gpsimd.wait_ge(tsem, 1)
                        nc.gpsimd.inc_swdge_sem(
                            [dsem], [gctr[0]], mode="sub"
                        ).then_inc(tsem, -1, skip_validation=True)
    nc.compile()
    return nc


def _prep_inputs(embs, ws, plan, mode=MODE):
    wcat = np.zeros((P, NCHUNK * D), _BF16)
    for b in (2, 3):
        for c in range(KS[b]):
            wcat[:, (WOFF[b] + c) * D : (WOFF[b] + c + 1) * D] = ws[b][
                c * P : (c + 1) * P, :
            ].astype(_BF16)

    def fold(emb, w, kc):  # pack (emb @ w) row-chunk-major: [P, kc*D]
        p = emb.astype(np.float32) @ w.astype(np.float32)
        ppad = np.zeros((kc * P, D), np.float32)
        ppad[: p.shape[0]] = p
        out = np.zeros((P, kc * D), _BF16)
        for c in range(kc):
            out[:, c * D : (c + 1) * D] = ppad[c * P : (c + 1) * P].astype(_BF16)
        return out

    p0c = fold(embs[0], ws[0], K0)
    p1full = embs[1].astype(np.float32) @ ws[1].astype(np.float32)  # [2700, D]
    k1 = plan.wrows[1] // P

    ident = np.eye(P, dtype=np.float32).astype(_BF16)

    e3bf = embs[3].astype(_BF16)
    e2f8 = embs[2].astype(_F8)

    def onehot(li, kc):
        oh = np.zeros((P, kc, P), np.float32)
        for t in range(li.shape[0]):
            r = int(li[t])
            oh[r % P, r // P, t] = 1.0
        return oh.reshape(P, kc * P).astype(_BF16)

    in_maps = []
    for c in range(NCORES):
        base = int(plan.wbase[3, c])
        w = plan.wrows[3]
        win = e3bf[base : base + w]
        if win.shape[0] < w:  # window runs past the table end: zero-pad
            win = np.concatenate([win, np.zeros((w - win.shape[0], DS[3]), _BF16)])
        b1b = int(plan.wbase[1, c])
        p1pad = np.zeros((k1 * P, D), np.float32)
        p1win = p1full[b1b : b1b + k1 * P]
        p1pad[: p1win.shape[0]] = p1win
        p1w = np.zeros((P, k1 * D), _BF16)
        for cc in range(k1):
            p1w[:, cc * D : (cc + 1) * D] = p1pad[cc * P : (cc + 1) * P].astype(_BF16)
        m = {
            "e3": np.ascontiguousarray(win),
            "e2": e2f8,
            "wcat": wcat,
            "p0c": p0c,
            "oh0": onehot(plan.li0[c], K0),
            "p1c": p1w,
            "oh1": onehot(plan.li1[c], k1),
            "ident": ident,
            "gidx": np.ascontiguousarray(plan.gidx[c]),
        }
        in_maps.append(m)
    return in_maps


def _assemble(plan, mode, results, repeat=1):
    out = np.empty((NTOK, D), np.float32)
    for c in range(NCORES):
        r = results[c]["out"]  # [128, T, D] partition-major
        r = np.ascontiguousarray(r.transpose(1, 0, 2)).reshape(-1, D)
        valid = plan.rowpos[c] >= 0
        out[plan.rowpos[c][valid]] = r[valid].astype(np.float32)
    return out.reshape(NCORES, SEQ, D)


def run(inputs, mode=MODE, trace=False):
    x = np.asarray(inputs["x"])
    embs = [np.asarray(inputs[f"emb{b}"]) for b in range(4)]
    ws = [np.asarray(inputs[f"W{b}"]) for b in range(4)]
    assert x.shape == (NCORES, SEQ), x.shape

    plan = _plan(x)
    key = (tuple(plan.alloc), tuple(plan.wrows), mode)
    if key not in _cache:
        _cache[key] = _build(plan, mode)
    nc = _cache[key]

    in_maps = _prep_inputs(embs, ws, plan, mode)
    res = run_bass_kernel_spmd(
        nc, in_maps, core_ids=list(range(NCORES)), trace=trace
    )
    out = _assemble(plan, mode, res.results)
    return out, res


def kernel(**inputs):
    out, _ = run(inputs, mode=MODE, trace=False)
    return out
